# revision 2
# baseline (speedup 1.0000x reference)
"""RWKV WKV recurrence kernel for Trainium2 (8 NeuronCores).

Math: for each (batch, channel) pair, over time t (w = -exp(time_decay) < 0,
u = time_first):
    A_t = e^w A_{t-1} + e^{k_t} v_t          (A_0 = 0)
    B_t = e^w B_{t-1} + e^{k_t}
    out_t = (A_{t-1} + e^{u+k_t} v_t) / (B_{t-1} + e^{u+k_t})
The reference uses a log-sum-exp-stabilized form of the same recurrence; for
these inputs (k ~ N(0,1), strictly negative decay) the state is geometrically
bounded so the direct fp32 form matches to ~5e-5 relative.

Mapping: batch (8) -> one NeuronCore each. Per core, channels go on SBUF
partitions (16 groups of 128) and time along the free dimension; the whole
T=2048 recurrence per group is one DVE tensor_tensor_scan (~1.8 cyc/elem).
num/den assembly uses a fused one-pass custom DVE op (out = in1 + s0*in0,
per-partition s0 = e^u), and the divide is a fused custom DVE op
(bitwise-NOT reciprocal seed + deg-2 minimax polynomial + multiply).

Structure: software-pipelined emission — LOAD(g) (super-group DMA with 1KB
lines + PE transposes + ScalarE exp / PSUM->SBUF copy) is emitted one group
ahead of COMP(g) (DVE scans/axpy/div + transpose-back + store), so every
engine queue holds work for two adjacent groups and a single pass pipelines.
SBUF work tiles are aliased (num<-vts, outg<-ekv, ost<-ek) to fit 2-deep
double buffering.
"""

import os
import sys
from contextlib import ExitStack

import numpy as np

for _p in ("/opt/trn_rl_repo", "/root/.axon_site/_ro/trn_rl_repo"):
    if os.path.isdir(_p) and _p not in sys.path:
        sys.path.insert(0, _p)

import concourse.bacc as bacc
import concourse.mybir as mybir
import concourse.tile as tile
from concourse import dve_ops as _dve_ops
from concourse import masks
from concourse.bass_utils import run_bass_kernel_spmd
from concourse.dve_spec import (
    Spec as _Spec,
    Src0,
    Src1,
    C0,
    C1,
    C2,
    Bin as _Bin,
    lower as _dve_lower,
)
from concourse.dve_uop import AluOp as _AluOp, DveOpSpec as _DveOpSpec

F32 = mybir.dt.float32
AF = mybir.ActivationFunctionType
OP = mybir.AluOpType

B, T, H = 8, 2048, 2048
N_CORES = 8

# out = num/den in ONE DVE pass: bitwise-NOT reciprocal seed
# (x*bitcast(~x) lands in [-4.5,-4]) + minimax deg-2 polynomial + multiply.
# Relative error ~5.1e-5.
_DIV_C0, _DIV_C1, _DIV_C2 = -0.7071067, -0.1665221, -0.013060556


def _register(name, spec, rd1_en=True):
    if name in _dve_ops._SUB_OPCODE_FOR_NAME:
        return next(o for o in _dve_ops.OPS if o.name == name)
    shas = {}
    for ver in ("v3", "v4"):
        try:
            uops = _dve_lower(spec, ver=ver)
        except Exception:
            continue
        shas[ver] = _DveOpSpec(name=name, opcode=0, uops=uops, rd1_en=rd1_en).sha(ver)
    op = _dve_ops.DveOp(name, spec, subdim=False, uops_sha=shas)
    row = _dve_ops._CUSTOM_DVE_ROW_BASE + len(_dve_ops.OPS)
    assert row < 0x20
    _dve_ops.OPS.append(op)
    _dve_ops._SUB_OPCODE_FOR_NAME[name] = row
    _dve_ops.CUSTOM_DVE_SPECS[name] = spec
    return op


def _div_mul_ref(in0, in1, c0, c1, c2):
    in0 = np.asarray(in0, np.float32)
    in1 = np.asarray(in1, np.float32)
    n = (~in0.view(np.int32)).view(np.float32)
    s = (in0 * n).astype(np.float32)
    q = (in1 * n).astype(np.float32)
    u = (s * np.float32(c2)).astype(np.float32)
    v = (np.float32(c1) + u).astype(np.float32)
    w = (s * v).astype(np.float32)
    p = (np.float32(c0) + w).astype(np.float32)
    return (q * p).astype(np.float32)


def _mk_div():
    _n = _Bin(_AluOp.BITWISE_NOT, Src0, Src0)
    _s = Src0 * _n
    body = (Src1 * _n) * (C0 + _s * (C1 + _s * C2))
    return _register("WKV_DIV_MUL_ANT", _Spec(body=body, reference=_div_mul_ref))


def _axpy_ref(in0, in1, c0):
    return (
        np.asarray(in1, np.float32)
        + np.asarray(c0, np.float32) * np.asarray(in0, np.float32)
    ).astype(np.float32)


def _mk_axpy():
    return _register("WKV_AXPY_ANT", _Spec(body=Src1 + C0 * Src0, reference=_axpy_ref))


WKV_DIV_MUL = _mk_div()
WKV_AXPY = _mk_axpy()


def build_nc(t=T, h=H, repeat=1, ablate=(), sg=2, ekv_engine="dve"):
    """Single-core program (SPMD across cores via differing inputs).

    repeat>1 duplicates the whole compute loop (same outputs) - timing only.
    ablate: stage names to skip (timing experiments; output wrong):
      {"scan", "stt", "recip", "outmul", "ekv", "inpath", "outpath"}.
    sg: channel groups per super-group input DMA (per-partition line = sg*512B).
    """
    ab = set(ablate)
    nc = bacc.Bacc("TRN2", target_bir_lowering=False, debug=False)

    key = nc.dram_tensor("key", [t, h], F32, kind="ExternalInput").ap()
    value = nc.dram_tensor("value", [t, h], F32, kind="ExternalInput").ap()
    td = nc.dram_tensor("time_decay", [h], F32, kind="ExternalInput").ap()
    tf = nc.dram_tensor("time_first", [h], F32, kind="ExternalInput").ap()
    out = nc.dram_tensor("out", [t, h], F32, kind="ExternalOutput").ap()

    G = h // 128
    SB = 512
    NB = t // SB
    W = sg * 128

    with tile.TileContext(nc) as tc, ExitStack() as ctx:
        const = ctx.enter_context(tc.tile_pool(name="const", bufs=1))
        identity = const.tile([128, 128], F32)
        masks.make_identity(nc, identity[:])

        tf_t = const.tile([128, G], F32)
        nc.sync.dma_start(tf_t[:], tf.rearrange("(g p) -> p g", p=128))
        td_t = const.tile([128, G], F32)
        nc.sync.dma_start(td_t[:], td.rearrange("(g p) -> p g", p=128))
        eu_t = const.tile([128, G], F32)
        nc.scalar.activation(eu_t[:], tf_t[:], AF.Exp)
        etd_t = const.tile([128, G], F32)
        nc.scalar.activation(etd_t[:], td_t[:], AF.Exp)
        ew_t = const.tile([128, G], F32)  # e^w = exp(-exp(td))
        nc.scalar.activation(ew_t[:], etd_t[:], AF.Exp, scale=-1.0)

        stage = ctx.enter_context(tc.tile_pool(name="stage", bufs=2))
        psin = ctx.enter_context(tc.tile_pool(name="psin", bufs=2, space="PSUM"))
        psout = ctx.enter_context(tc.tile_pool(name="psout", bufs=2, space="PSUM"))
        grp = ctx.enter_context(tc.tile_pool(name="grp", bufs=2))

        seq = [gg for _ in range(repeat) for gg in range(G)]
        sg_tiles = {}

        def load(gi):
            if "inpath" in ab:
                return None
            sgi = gi // sg
            if sgi not in sg_tiles:
                g0 = seq[sgi * sg]
                hsg = slice(g0 * 128, g0 * 128 + W)
                kc = stage.tile([128, (t // 128) * W], F32, tag="kc")
                nc.sync.dma_start(
                    kc[:].rearrange("p (s w) -> p s w", w=W),
                    key[:, hsg].rearrange("(s p) w -> p s w", p=128),
                )
                vc = stage.tile([128, (t // 128) * W], F32, tag="vc")
                nc.scalar.dma_start(
                    vc[:].rearrange("p (s w) -> p s w", w=W),
                    value[:, hsg].rearrange("(s p) w -> p s w", p=128),
                )
                sg_tiles[sgi] = (kc, vc)
                if len(sg_tiles) > 2:
                    del sg_tiles[min(sg_tiles)]
            kc, vc = sg_tiles[sgi]
            goff = (gi % sg) * 128

            ek = grp.tile([128, t], F32, tag="ek")
            vts = grp.tile([128, t], F32, tag="vts")
            for nb in range(NB):
                kT = psin.tile([128, SB], F32, tag="kT")
                vT = psin.tile([128, SB], F32, tag="vT")
                for c in range(SB // 128):
                    s = nb * (SB // 128) + c
                    src = slice(s * W + goff, s * W + goff + 128)
                    cs = slice(c * 128, (c + 1) * 128)
                    nc.tensor.transpose(kT[:, cs], kc[:, src], identity[:])
                    nc.tensor.transpose(vT[:, cs], vc[:, src], identity[:])
                bsl = slice(nb * SB, (nb + 1) * SB)
                nc.scalar.activation(ek[:, bsl], kT[:], AF.Exp)
                nc.scalar.copy(vts[:, bsl], vT[:])
            return ek, vts

        def comp(gi, tiles):
            g = seq[gi]
            eu_g = eu_t[:, g : g + 1]
            ew_g = ew_t[:, g : g + 1]
            hs = slice(g * 128, (g + 1) * 128)
            if tiles is None:
                ek = grp.tile([128, t], F32, tag="ek")
                vts = grp.tile([128, t], F32, tag="vts")
            else:
                ek, vts = tiles

            ekv = grp.tile([128, t], F32, tag="ekv")
            A = grp.tile([128, t + 1], F32, tag="A")
            Bb = grp.tile([128, t + 1], F32, tag="B")
            den = grp.tile([128, t], F32, tag="den")
            num = vts  # alias: vts dead once ekv computed
            outg = ekv  # alias: ekv dead once num computed
            ost = ek  # alias: ek dead once den computed

            d0 = ew_g.broadcast_to((128, t))
            # scanB first so the DVE has work while ekv is produced
            if "scan" not in ab:
                nc.vector.memset(Bb[:, 0:1], 0.0)
                nc.vector.tensor_tensor_scan(
                    Bb[:, 1 : t + 1], d0, ek[:], 0.0, OP.mult, OP.add
                )
            if "ekv" not in ab:
                if ekv_engine == "gpsimd":
                    nc.gpsimd.tensor_mul(ekv[:], ek[:], vts[:])
                else:
                    nc.vector.tensor_mul(ekv[:], ek[:], vts[:])
            src_t = ekv if "ekv" not in ab else ek
            if "scan" not in ab:
                nc.vector.memset(A[:, 0:1], 0.0)
                nc.vector.tensor_tensor_scan(
                    A[:, 1 : t + 1], d0, src_t[:], 0.0, OP.mult, OP.add
                )
                A_r, B_r = A[:, 0:t], Bb[:, 0:t]
            else:
                A_r, B_r = src_t[:], ek[:]

            # num = A_{t-1} + e^u*ekv ; den = B_{t-1} + e^u*ek  (one pass each)
            if "stt" not in ab:
                nc.vector._custom_dve(WKV_AXPY, out=den[:], in0=ek[:], in1=B_r, s0=eu_g)
                nc.vector._custom_dve(
                    WKV_AXPY, out=num[:], in0=src_t[:], in1=A_r, s0=eu_g
                )
                num_r, den_r = num, den
            else:
                num_r, den_r = src_t, ek

            if "recip" in ab or "outmul" in ab:
                outg_r = num_r
            else:
                nc.vector._custom_dve(
                    WKV_DIV_MUL,
                    out=outg[:],
                    in0=den_r[:],
                    in1=num_r[:],
                    s0=_DIV_C0,
                    s1=_DIV_C1,
                    imm2=_DIV_C2,
                )
                outg_r = outg

            if "outpath" not in ab:
                for nb in range(NB):
                    oT = psout.tile([128, SB], F32, tag="oT")
                    for c in range(SB // 128):
                        s = nb * (SB // 128) + c
                        nc.tensor.transpose(
                            oT[:, c * 128 : (c + 1) * 128],
                            outg_r[:, s * 128 : (s + 1) * 128],
                            identity[:],
                        )
                    nc.scalar.copy(ost[:, nb * SB : (nb + 1) * SB], oT[:])
                    nc.sync.dma_start(
                        out[nb * SB : (nb + 1) * SB, hs].rearrange(
                            "(s p) h -> p s h", p=128
                        ),
                        ost[:, nb * SB : (nb + 1) * SB].rearrange(
                            "p (s h) -> p s h", h=128
                        ),
                    )

        tiles = load(0)
        for gi in range(len(seq)):
            nxt = load(gi + 1) if gi + 1 < len(seq) else None
            comp(gi, tiles)
            tiles = nxt

    nc.compile()
    return nc


_nc_cache = {}


def _get_nc():
    if "nc" not in _nc_cache:
        _nc_cache["nc"] = build_nc()
    return _nc_cache["nc"]


def kernel_with_results(key, value, time_decay, time_first, trace=False):
    nc = _get_nc()
    key = np.ascontiguousarray(key, dtype=np.float32)
    value = np.ascontiguousarray(value, dtype=np.float32)
    time_decay = np.ascontiguousarray(time_decay, dtype=np.float32)
    time_first = np.ascontiguousarray(time_first, dtype=np.float32)
    in_maps = [
        {
            "key": key[i],
            "value": value[i],
            "time_decay": time_decay,
            "time_first": time_first,
        }
        for i in range(N_CORES)
    ]
    res = run_bass_kernel_spmd(nc, in_maps, list(range(N_CORES)), trace=trace)
    out = np.stack([res.results[i]["out"] for i in range(N_CORES)], axis=0)
    return out, res


def kernel(key, value, time_decay, time_first):
    out, _ = kernel_with_results(key, value, time_decay, time_first)
    return out
